# revision 4
# baseline (speedup 1.0000x reference)
"""Trainium2 Bass kernel for nn_DeformRouting (deformable routing conv).

Strategy (8 cores, data-parallel over N x H-halves):
  core c handles image n = c//2, row-half = c%2 (14 rows x 28 cols = 392 pixels).

Pipeline (~62us vs 156us for the v1 two-f32-gather kernel):
  - ONE merged bf16 gather per 98-point chunk over a 30x30 2x2-PATCH table
    (each 512B row = all 4 bilinear corners x 64 channels), 4 chunks of
    1152 indices pipelined on 2 SWDGE queues; tiny warm-up-free launch.
  - zero-padded patch table kills all validity math: out-of-range corners
    fetch zeros, so only the 4 plain bilinear weights are needed.
  - fused coordinate math (~12 DVE ops): the +1 table shift, offset bias,
    and grid constants are host-folded into base1 so
    i2 = clamp(off*13.5 + base1, 0, <29) directly indexes the table.
  - corner weights pre-expanded along c on the otherwise-idle ACT engine
    so the combine multiplies run in DVE 2x (packed bf16) mode.
  - k-major weight-matrix reorder (host) keeps combine writes contiguous;
    samp padded to 10 k-slots = exactly 5 PE transpose blocks.
  - per-chunk final matmuls/output so the tail after the last gather chunk
    is only one chunk deep; bf16 on all wide paths, f32 coordinates.
"""

import ml_dtypes
import numpy as np

import concourse.bass as bass
import concourse.tile as tile
from concourse import bacc, mybir
from concourse.bass_utils import run_bass_kernel_spmd
from concourse.masks import make_identity

# problem constants (hardcoded per contract)
N, CIN, COUT, H, W, K = 4, 64, 64, 28, 28, 3
K2 = K * K  # 9
NCORES = 8
HHALF = H // 2          # 14 rows per core
NPT = HHALF * W         # 392 points per core
PCH = 98                # points per partition-chunk
NCH = 4                 # chunks (4*98 = 392)
SC = (W - 1) / 2.0      # 13.5
TDIM = H + 2            # 30: patch-table rows/cols (sy, sx in [0, 30))
TROWS = TDIM * TDIM     # 900 table rows
ELEM = 4 * CIN          # 256 bf16 per table row (4 corners x 64 ch)
KP = 10                 # k padded 9 -> 10 so 2*KP*CIN/128 = 5 blocks
NB = 5                  # transpose/matmul blocks per tensor
CLIP_HI = float(np.nextafter(np.float32(TDIM - 1), np.float32(0.0)))  # 28.999998

F32 = mybir.dt.float32
BF16 = mybir.dt.bfloat16
I32 = mybir.dt.int32
I16 = mybir.dt.int16

_CACHE = {}


def _alu(name):
    return getattr(mybir.AluOpType, name)


def _build_program(nq=2):
    """Build + compile the (SPMD-identical) Bass program once."""
    nc = bacc.Bacc("TRN2", target_bir_lowering=False, debug=False,
                   num_devices=NCORES, num_swdge_queues=nq,
                   dynamic_dma_scratch_size=65536)

    # DRAM I/O (per-core shapes)
    xpatch = nc.dram_tensor("xpatch", [TROWS, ELEM], BF16, kind="ExternalInput")
    xcpad = nc.dram_tensor("xcpad", [128, NPT], BF16, kind="ExternalInput")
    xout = nc.dram_tensor("xout", [COUT, NPT], F32, kind="ExternalInput")
    wofft = nc.dram_tensor("wofft", [128, 2 * K2], BF16, kind="ExternalInput")
    base1 = nc.dram_tensor("base1", [128, NCH * 2 * K2], F32, kind="ExternalInput")
    wwb = nc.dram_tensor("wwb", [128, 2 * NB * COUT], BF16, kind="ExternalInput")
    mg = nc.dram_tensor("mg", [128, 8 * 128], BF16, kind="ExternalInput")
    out_d = nc.dram_tensor("out", [COUT, NPT], F32, kind="ExternalOutput")

    mult, add, sub = _alu("mult"), _alu("add"), _alu("subtract")
    amin, amax = _alu("min"), _alu("max")

    with tile.TileContext(nc) as tc:
        with (
            tc.tile_pool(name="const", bufs=1) as cpool,
            tc.tile_pool(name="work", bufs=1) as wpool,
            tc.tile_pool(name="psoff", bufs=1, space="PSUM") as opool,
            tc.tile_pool(name="pst", bufs=2, space="PSUM") as ppool,
            tc.tile_pool(name="pso", bufs=1, space="PSUM") as popool,
        ):
            # ---- load constants/inputs (issue split across the three
            # HWDGE-capable engines; each dma_start costs ~1.2us of its
            # issuing sequencer, so serializing them on SP starves the conv)
            xc_sb = cpool.tile([128, NPT], BF16)
            nc.sync.dma_start(xc_sb[:], xcpad.ap())
            wofft_sb = cpool.tile([128, 2 * K2], BF16)
            nc.sync.dma_start(wofft_sb[:], wofft.ap())
            base1_sb = cpool.tile([128, NCH, 2 * K2], F32)
            nc.sync.dma_start(base1_sb[:], base1.ap().rearrange(
                "p (a b) -> p a b", a=NCH))
            mg_sb = cpool.tile([128, 8, 128], BF16)
            nc.sync.dma_start(mg_sb[:], mg.ap().rearrange(
                "p (a b) -> p a b", a=8))
            wwb_sb = cpool.tile([128, 2 * NB, COUT], BF16)
            nc.sync.dma_start(wwb_sb[:], wwb.ap().rearrange(
                "p (a b) -> p a b", a=2 * NB))
            xout_sb = cpool.tile([COUT, NPT], F32)
            nc.sync.dma_start(xout_sb[:], xout.ap())
            ident = cpool.tile([128, 128], BF16)
            make_identity(nc, ident[:])

            # ---- 1. offset conv: ps_off[pt(98), ch, 18] (x offs 0:9, y 9:18)
            ps_off = opool.tile([128, NCH, 2 * K2], F32)
            nc.any.memset(ps_off[:], 0.0)
            for ch in range(NCH):
                nc.tensor.matmul(
                    out=ps_off[:PCH, ch, :],
                    lhsT=xc_sb[:, ch * PCH:(ch + 1) * PCH],
                    rhs=wofft_sb[:],
                    start=True, stop=True,
                )

            # ---- 2. coordinate math on [128, NCH, 18] (x cols 0:9, y 9:18)
            shp = [128, NCH, 2 * K2]
            t1 = wpool.tile(shp, F32, name="t1")
            nc.vector.scalar_tensor_tensor(t1[:], ps_off[:], SC, base1_sb[:],
                                           mult, add)
            i2 = wpool.tile(shp, F32, name="i2")
            nc.vector.tensor_scalar(i2[:], t1[:], CLIP_HI, 0.001, amin, amax)
            # floor via round-to-nearest(i2 - 0.5): the i32 cast rounds, and
            # at exact-integer ties either choice yields the same bilinear
            # sample (weight 1 on the shared corner). i2 >= 0.001 keeps the
            # result provably non-negative.
            si = wpool.tile(shp, I32, name="si")
            nc.vector.tensor_scalar(si[:], i2[:], -0.5, 0.0, add, add)
            sf = wpool.tile(shp, F32, name="sf")
            nc.vector.tensor_copy(sf[:], si[:])
            sfb = wpool.tile(shp, BF16, name="sfb")      # exact: ints <= 28
            nc.any.tensor_copy(sfb[:], sf[:])
            w1 = wpool.tile(shp, F32, name="w1")
            nc.vector.tensor_tensor(w1[:], i2[:], sf[:], sub)
            w0 = wpool.tile(shp, F32, name="w0")
            nc.vector.tensor_scalar(w0[:], w1[:], -1.0, 1.0, mult, add)

            # corner weights [128, NCH, 9] bf16; y slice 9:18, x slice 0:9
            wshp = [128, NCH, K2]
            wTL = wpool.tile(wshp, BF16, name="wTL")
            wTR = wpool.tile(wshp, BF16, name="wTR")
            wBL = wpool.tile(wshp, BF16, name="wBL")
            wBR = wpool.tile(wshp, BF16, name="wBR")
            nc.vector.tensor_tensor(wTL[:], w0[:, :, K2:], w0[:, :, :K2], mult)
            nc.vector.tensor_tensor(wTR[:], w0[:, :, K2:], w1[:, :, :K2], mult)
            nc.vector.tensor_tensor(wBL[:], w1[:, :, K2:], w0[:, :, :K2], mult)
            nc.vector.tensor_tensor(wBR[:], w1[:, :, K2:], w1[:, :, :K2], mult)
            # pre-expand corner weights along c on the (otherwise idle) ACT
            # engine so the combine multiplies run in DVE 2x mode
            wx = []
            for wi, wt in enumerate((wTL, wTR, wBL, wBR)):
                we = wpool.tile([128, NCH, K2, CIN], BF16, name=f"wx{wi}")
                nc.scalar.copy(
                    we[:], wt[:, :, :, None].to_broadcast([128, NCH, K2, CIN]))
                wx.append(we)

            # ---- 3. index wrap into dma_gather's 16-partition layout:
            # psw[q, g, (a, 18)] = sfb[g*16 + q%16, (a, 18)] via 8 perm matmuls,
            # then wrap[q, (a,k), g] = 30*psw_y + psw_x as int16.
            psw0 = opool.tile([128, 4, NCH * 2 * K2], F32, name="psw0")
            psw1 = opool.tile([128, 4, NCH * 2 * K2], F32, name="psw1")
            for g in range(8):
                ps = psw0 if g < 4 else psw1
                nc.tensor.matmul(
                    out=ps[:, g % 4, :], lhsT=mg_sb[:, g, :],
                    rhs=sfb[:].rearrange("p a b -> p (a b)"),
                    start=True, stop=True)
            psw_sb = wpool.tile([128, 8, NCH, 2 * K2], F32, name="psw_sb")
            nc.any.tensor_copy(
                psw_sb[:, 0:4, :, :],
                psw0[:].rearrange("q g (a b) -> q g a b", a=NCH))
            nc.any.tensor_copy(
                psw_sb[:, 4:8, :, :],
                psw1[:].rearrange("q g (a b) -> q g a b", a=NCH))
            wrap = wpool.tile([128, NCH * K2, 8], I16, name="wrap")
            nc.vector.scalar_tensor_tensor(
                wrap[:].rearrange("q (a k) g -> q g a k", a=NCH),
                psw_sb[:, :, :, K2:], float(TDIM), psw_sb[:, :, :, :K2],
                mult, add)

            # ---- 4-6. per-chunk: gather -> bilinear combine -> PE transpose
            rhs = wpool.tile([128, NB, NPT], BF16, name="rhs")
            ps1 = popool.tile([COUT, NPT], F32, name="ps1")
            ps2 = popool.tile([COUT, NPT], F32, name="ps2")
            out_sb = wpool.tile([COUT, NPT], F32, name="out_sb")
            NI = K2 * 128  # 1152 indices per chunk

            for a in range(NCH):
                gx = wpool.tile([128, K2, 4, CIN], BF16, name=f"gx{a}")
                nc.gpsimd.dma_gather(
                    out_ap=gx[:].rearrange("p k j c -> p k (j c)"),
                    in_ap=xpatch.ap(),
                    idxs_ap=wrap[:, a * K2:(a + 1) * K2, :].rearrange(
                        "q m g -> q (m g)"),
                    num_idxs=NI, num_idxs_reg=NI, elem_size=ELEM,
                    single_packet=False, queue_num=a % nq)

                s_t = wpool.tile([128, KP, CIN], BF16, name=f"s{a}")
                nc.any.memset(s_t[:, K2, :], 0.0)
                sa = s_t[:, :K2, :]

                tA = wpool.tile([128, K2, CIN], BF16, name=f"tA{a}")
                tB = wpool.tile([128, K2, CIN], BF16, name=f"tB{a}")
                nc.vector.tensor_tensor(sa, gx[:, :, 0, :], wx[0][:, a], mult)
                nc.vector.tensor_tensor(tA, gx[:, :, 1, :], wx[1][:, a], mult)
                nc.vector.tensor_tensor(sa, sa, tA, add)
                nc.vector.tensor_tensor(tB, gx[:, :, 2, :], wx[2][:, a], mult)
                nc.vector.tensor_tensor(sa, sa, tB, add)
                nc.vector.tensor_tensor(tA, gx[:, :, 3, :], wx[3][:, a], mult)
                nc.vector.tensor_tensor(sa, sa, tA, add)

                sv = s_t[:].rearrange("p k c -> p (k c)")
                for b in range(NB):
                    pst = ppool.tile([128, 128], BF16, tag="tps")
                    nc.tensor.transpose(
                        pst[:], sv[:, 128 * b:128 * (b + 1)], ident[:])
                    nc.any.tensor_copy(
                        rhs[:, b, a * PCH:(a + 1) * PCH], pst[:, :PCH])

                # ---- 7. per-chunk final matmuls + output columns ----
                cols = slice(a * PCH, (a + 1) * PCH)
                for b in range(NB):
                    nc.tensor.matmul(
                        out=ps1[:, cols], lhsT=wwb_sb[:, b, :],
                        rhs=rhs[:, b, cols],
                        start=(b == 0), stop=(b == NB - 1))
                for b in range(NB):
                    nc.tensor.matmul(
                        out=ps2[:, cols], lhsT=wwb_sb[:, NB + b, :],
                        rhs=rhs[:, b, cols],
                        start=(b == 0), stop=(b == NB - 1))
                nc.vector.tensor_tensor(out_sb[:, cols], ps1[:, cols],
                                        xout_sb[:, cols], mult)
                nc.vector.tensor_tensor(out_sb[:, cols], out_sb[:, cols],
                                        ps2[:, cols], add)
                nc.sync.dma_start(out_d.ap()[:, cols], out_sb[:, cols])

    nc.compile()
    return nc


def _host_inputs(x, w_off, b_off, w_wgt, b_wgt):
    """Build the 8 per-core input dicts (layout/shard prep only)."""
    x = np.asarray(x, dtype=np.float32)
    w_off = np.asarray(w_off, dtype=np.float32)
    b_off = np.asarray(b_off, dtype=np.float32)
    w_wgt = np.asarray(w_wgt, dtype=np.float32)
    b_wgt = np.asarray(b_wgt, dtype=np.float32)

    xs = np.linspace(-1.0, 1.0, W).astype(np.float32)
    ys = np.linspace(-1.0, 1.0, H).astype(np.float32)
    kx = np.linspace(-(K - 1) / (W - 1), (K - 1) / (W - 1), K).astype(np.float32)
    ky = np.linspace(-(K - 1) / (H - 1), (K - 1) / (H - 1), K).astype(np.float32)

    # wwb [128, 10, 64]: blocks 0..4 = W~ k-major (rows m = kk*64 + c,
    # kk in [0,10) with kk=9 zero), blocks 5..9 = B~ k-major.
    # W~km[kk*64+c, o] = w_wgt[o, c*9+kk]; B~km likewise from b_wgt.
    W4 = w_wgt.reshape(COUT, CIN, K2)          # [o, c, kk]
    B4 = b_wgt.reshape(COUT, CIN, K2)          # [o, c, kk]
    Wkm = np.zeros((KP, CIN, COUT), dtype=np.float32)
    Bkm = np.zeros((KP, CIN, COUT), dtype=np.float32)
    Wkm[:K2] = W4.transpose(2, 1, 0)
    Bkm[:K2] = B4.transpose(2, 1, 0)
    wwb = np.concatenate([Wkm.reshape(NB, 128, COUT),
                          Bkm.reshape(NB, 128, COUT)], axis=0)  # [10,128,64]
    wwb = wwb.transpose(1, 0, 2).reshape(128, 2 * NB * COUT)

    # idx-wrap permutation selectors: mg[g*16 + q%16, g, q] = 1
    mg = np.zeros((128, 8, 128), dtype=np.float32)
    q = np.arange(128)
    for g in range(8):
        mg[g * 16 + (q % 16), g, q] = 1.0
    mg = mg.reshape(128, 8 * 128)

    # offset conv weights, column-reordered: x offsets 0:9, y offsets 9:18
    wofft = np.zeros((128, 2 * K2), dtype=np.float32)
    wofft[:CIN, :K2] = w_off[0::2].T
    wofft[:CIN, K2:] = w_off[1::2].T

    in_maps = []
    tables = {}
    for c in range(NCORES):
        n, half = divmod(c, 2)
        r0 = HHALF * half
        xn = x[n]                             # [64, 28, 28]

        if n not in tables:
            xz = np.zeros((CIN, TDIM + 1, TDIM + 1), dtype=np.float32)
            xz[:, 1:H + 1, 1:W + 1] = xn
            P = np.empty((TDIM, TDIM, 4, CIN), dtype=np.float32)
            P[:, :, 0, :] = xz[:, 0:TDIM, 0:TDIM].transpose(1, 2, 0)
            P[:, :, 1, :] = xz[:, 0:TDIM, 1:TDIM + 1].transpose(1, 2, 0)
            P[:, :, 2, :] = xz[:, 1:TDIM + 1, 0:TDIM].transpose(1, 2, 0)
            P[:, :, 3, :] = xz[:, 1:TDIM + 1, 1:TDIM + 1].transpose(1, 2, 0)
            tables[n] = P.reshape(TROWS, ELEM)

        xcpad = np.zeros((128, NPT), dtype=np.float32)
        xcpad[:CIN] = xn.reshape(CIN, H * W)[:, r0 * W:r0 * W + NPT]

        # base1 [128, NCH, 18]: i2 = off*13.5 + base1 directly indexes the
        # padded table: base1 = (grid + b_off + 1)*13.5 + 1
        b1 = np.full((128, NCH, 2 * K2), 14.5, dtype=np.float32)
        p_idx = np.arange(PCH)
        for ch in range(NCH):
            g = r0 * W + ch * PCH + p_idx          # global pixel
            row, col = g // W, g % W
            for kk in range(K2):
                kyi, kxi = divmod(kk, K)
                b1[:PCH, ch, kk] = (xs[col] + kx[kxi] + b_off[2 * kk]
                                    + 1.0) * SC + 1.0
                b1[:PCH, ch, K2 + kk] = (ys[row] + ky[kyi] + b_off[2 * kk + 1]
                                         + 1.0) * SC + 1.0

        bf = ml_dtypes.bfloat16
        in_maps.append({
            "xpatch": tables[n].astype(bf),
            "xcpad": xcpad.astype(bf),
            "xout": xcpad[:COUT].copy(),
            "wofft": wofft.astype(bf),
            "base1": b1.reshape(128, NCH * 2 * K2),
            "wwb": wwb.astype(bf),
            "mg": mg.astype(bf),
        })
    return in_maps


def get_program():
    if "nc" not in _CACHE:
        _CACHE["nc"] = _build_program()
    return _CACHE["nc"]


def run_cores(in_maps, **kw):
    nc = get_program()
    return run_bass_kernel_spmd(nc, in_maps, core_ids=list(range(NCORES)), **kw)


def assemble(results):
    out = np.zeros((N, COUT, H, W), dtype=np.float32)
    for c in range(NCORES):
        n, half = divmod(c, 2)
        out[n, :, HHALF * half:HHALF * (half + 1), :] = \
            results[c]["out"].reshape(COUT, HHALF, W)
    return out


def kernel(x, w_off, b_off, w_wgt, b_wgt):
    in_maps = _host_inputs(x, w_off, b_off, w_wgt, b_wgt)
    res = run_cores(in_maps)
    return assemble(res.results)
